# revision 1
# baseline (speedup 1.0000x reference)
"""GAT (2-layer, 4-head) Trainium2 Bass kernel, SPMD over 8 NeuronCores.

Sharding: 1D row partition of N. Each core computes its N/8-row block of
both attention layers. h (per-head projected features) is computed
replicated on every core (cheap); layer-2 features are exchanged with a
single AllGather.

Key tricks:
- Scores are computed in TRANSPOSED form p[j, i] (source node j on
  partitions, this core's destination rows i on the free axis), so the
  softmax contraction axis is the partition axis and no N x N transposes
  are ever needed on device (X and the mask are fed pre-transposed).
- The score exponential is SEPARABLE: with t = s1_i + s2_j + c,
  exp(leaky_relu(t)) = max(E1_i*F1_j, E2_i*F2_j) where E1=exp(s1+c),
  F1=exp(s2), E2=exp(0.2(s1+c)), F2=exp(0.2 s2). All transcendentals
  collapse to tiny per-head vectors; the N x N stream needs only
  mul/max ops (spread across ScalarE/VectorE/GpSimdE).
- Softmax denominators come free from the aggregation matmul via a
  ones-column prepended to the feature matrix (unnormalized exp, one
  reciprocal per output row at the end). Aggregation is emitted
  "natural-out" (rows on partitions), so the denominator is a [P,1]
  column and the divide is a trivial per-partition tensor_scalar.
- elu's -1, all biases, and the attention-vector projections are folded
  on the host into score constants / a post-normalize bias add.
"""

import numpy as np
import ml_dtypes
from contextlib import ExitStack

BF16 = ml_dtypes.bfloat16

# problem constants (hardcoded per contract)
N, FIN, FH, H, FO = 4096, 512, 128, 4, 64
N_CORES = 8
FCAT = H * FH  # 512
GRW = 2 + FO   # gather row width: [s2_o | ones-slot | h2 features]


def _cfg(n=N, n_cores=N_CORES):
    nb = n // n_cores        # rows per core
    return dict(
        n=n, nb=nb, n_cores=n_cores,
        jc=n // 128,          # j (source-node) chunks of 128
        kc=FIN // 128,        # contraction chunks for X @ W
        ic=(nb + 127) // 128, # i chunks for natural-out aggregation
    )


def host_prep(X, adjacency_matrix, W_h, b_h, a1_h, a2_h, ab_h,
              W_o, b_o, a1_o, a2_o, ab_o, cfg):
    """Fold weights / biases on the host; produce per-core input maps."""
    f32 = np.float32
    X = np.asarray(X, f32)
    adj = np.asarray(adjacency_matrix)
    W_h, b_h = np.asarray(W_h, f32), np.asarray(b_h, f32)
    a1_h, a2_h, ab_h = np.asarray(a1_h, f32), np.asarray(a2_h, f32), np.asarray(ab_h, f32)
    W_o, b_o = np.asarray(W_o, f32), np.asarray(b_o, f32)
    a1_o, a2_o, ab_o = np.asarray(a1_o, f32), np.asarray(a2_o, f32), np.asarray(ab_o, f32)

    nb, ncores = cfg["nb"], cfg["n_cores"]

    XT = np.ascontiguousarray(X.T).astype(BF16)                  # [FIN, n]
    maskT = np.ascontiguousarray((adj > 0).T.astype(f32)).astype(BF16)

    WC = np.concatenate([W_h[h] for h in range(H)], axis=1).astype(BF16)
    wa1 = np.stack([W_h[h] @ a1_h[h] for h in range(H)], axis=1)  # [FIN, H]
    wa2 = np.stack([W_h[h] @ a2_h[h] for h in range(H)], axis=1)  # [FIN, H]
    WA4 = wa2.astype(BF16)
    WA8 = np.concatenate([wa1, wa2], axis=1).astype(BF16)         # [FIN, 2H]
    c1 = np.array([b_h[h] @ a1_h[h] + ab_h[h] for h in range(H)], f32)
    c2 = np.array([b_h[h] @ a2_h[h] for h in range(H)], f32)
    c12 = (c1 + c2).astype(f32)

    wao1 = W_o @ a1_o
    wao2 = W_o @ a2_o
    # elu(-1) fold: device computes x_raw = elu(z)+1; corrections:
    c1o = float(b_o @ a1_o + ab_o - wao1.sum())
    c2o = float(b_o @ a2_o - wao2.sum())
    beta = (b_o - W_o.sum(axis=0)).astype(f32)                    # [FO]
    BETA = np.ascontiguousarray(np.broadcast_to(beta, (128, FO)))

    GR = np.concatenate(
        [wao2[:, None], np.zeros((FCAT, 1), f32), W_o], axis=1
    ).astype(BF16)                                                # [FCAT, GRW]
    WAO1 = wao1[:, None].astype(BF16)
    # b_h broadcast to all partitions, concatenated along heads
    BHB = np.ascontiguousarray(
        np.broadcast_to(np.concatenate([b_h[h] for h in range(H)]),
                        (128, FCAT))).astype(BF16)                # [128, H*FH]

    shared = {
        "XT": XT, "WC": WC, "WA4": WA4, "WA8": WA8,
        "GR": GR, "WAO1": WAO1, "BHB": BHB, "BETA": BETA,
    }
    in_maps = []
    for c in range(ncores):
        m = dict(shared)
        m["XTo"] = np.ascontiguousarray(XT[:, c * nb:(c + 1) * nb])
        m["MT"] = np.ascontiguousarray(maskT[:, c * nb:(c + 1) * nb])
        in_maps.append(m)
    return in_maps, {"c12": c12, "c12o": c1o + c2o}


def build_program(nc, tc, cfg, consts, skip_collective=False):
    """Emit the SPMD GAT program into TileContext tc."""
    from concourse import mybir

    f32 = mybir.dt.float32
    bf16 = mybir.dt.bfloat16
    AF = mybir.ActivationFunctionType
    OP = mybir.AluOpType
    AX = mybir.AxisListType

    n, nb, jcn, kcn, icn = cfg["n"], cfg["nb"], cfg["jc"], cfg["kc"], cfg["ic"]
    ncores = cfg["n_cores"]
    c12, c12o = consts["c12"], consts["c12o"]
    HCW = FH + 1  # 129: [ones | 128 features] per (head, jc)

    # ---- DRAM I/O ----
    d_XT = nc.dram_tensor("XT", [FIN, n], bf16, kind="ExternalInput")
    d_XTo = nc.dram_tensor("XTo", [FIN, nb], bf16, kind="ExternalInput")
    d_MT = nc.dram_tensor("MT", [n, nb], bf16, kind="ExternalInput")
    d_WC = nc.dram_tensor("WC", [FIN, FCAT], bf16, kind="ExternalInput")
    d_WA4 = nc.dram_tensor("WA4", [FIN, H], bf16, kind="ExternalInput")
    d_WA8 = nc.dram_tensor("WA8", [FIN, 2 * H], bf16, kind="ExternalInput")
    d_GR = nc.dram_tensor("GR", [FCAT, GRW], bf16, kind="ExternalInput")
    d_WAO1 = nc.dram_tensor("WAO1", [FCAT, 1], bf16, kind="ExternalInput")
    d_BHB = nc.dram_tensor("BHB", [128, FCAT], bf16, kind="ExternalInput")
    d_BETA = nc.dram_tensor("BETA", [128, FO], f32, kind="ExternalInput")
    d_OUT = nc.dram_tensor("OUT", [nb, FO], f32, kind="ExternalOutput")

    ctx = ExitStack()
    with ctx:
        cpool = ctx.enter_context(tc.tile_pool(name="const", bufs=1))
        work = ctx.enter_context(tc.tile_pool(name="work", bufs=6))
        spool = ctx.enter_context(tc.tile_pool(name="small", bufs=2))
        dpool = ctx.enter_context(tc.tile_pool(name="dram", bufs=1, space="DRAM"))

        def load(name, dram, parts, width, dt=bf16, rearr=True):
            t = cpool.tile([parts, width], dt, tag=name, name=f"ld_{name}")
            if rearr:
                src = dram.ap().rearrange("(c p) x -> p c x", p=parts)
                dst = t[:].rearrange("p (c x) -> p c x", c=src.shape[1])
                nc.sync.dma_start(dst, src)
            else:
                nc.sync.dma_start(t[:], dram.ap())
            return t

        XTo_sb = load("XTo", d_XTo, 128, kcn * nb)
        WA8_sb = load("WA8", d_WA8, 128, kcn * 2 * H)
        WC_sb = load("WC", d_WC, 128, kcn * FCAT)
        WA4_sb = load("WA4", d_WA4, 128, kcn * H)
        # XT / MT loaded in per-jc chunks so dependent compute starts early
        XT_sb = cpool.tile([128, kcn * n], bf16, tag="XT", name="ld_XT")
        for jc in range(jcn):
            src = d_XT.ap()[:, jc * 128:(jc + 1) * 128].rearrange(
                "(kc p) x -> p kc x", p=128)
            dst = XT_sb[:].rearrange("p (kc x) -> p kc x", kc=kcn)[
                :, :, jc * 128:(jc + 1) * 128]
            nc.sync.dma_start(dst, src)
        MT_sb = cpool.tile([128, jcn * nb], bf16, tag="MT", name="ld_MT")
        for jc in range(jcn):
            nc.sync.dma_start(MT_sb[:, jc * nb:(jc + 1) * nb],
                              d_MT.ap()[jc * 128:(jc + 1) * 128, :])
        GR_sb = load("GR", d_GR, 128, kcn * GRW)
        WAO1_sb = load("WAO1", d_WAO1, 128, kcn)
        BHB_sb = load("BHB", d_BHB, 128, FCAT, rearr=False)
        BETA_sb = load("BETA", d_BETA, 128, FO, dt=f32, rearr=False)

        onesb = cpool.tile([1, 128], bf16, tag="ones")
        nc.vector.memset(onesb[:], 1.0)

        # score-constant bias columns: [c12_h | 0.2*c12_h | c12o | 0.2*c12o]
        cb = cpool.tile([128, 2 * H + 2], f32, tag="cb")
        for hh in range(H):
            nc.vector.memset(cb[:, hh:hh + 1], float(c12[hh]))
            nc.vector.memset(cb[:, H + hh:H + hh + 1], float(0.2 * c12[hh]))
        nc.vector.memset(cb[:, 2 * H:2 * H + 1], float(c12o))
        nc.vector.memset(cb[:, 2 * H + 1:2 * H + 2], float(0.2 * c12o))

        # h_all: per (head, jc): [ones | 128 feature cols] on the free axis
        h_all = cpool.tile([128, H * jcn * HCW], bf16, tag="h_all")
        nc.vector.memset(h_all[:, 0:H * jcn * HCW:HCW], 1.0)  # ones columns

        s2colT = cpool.tile([128, jcn * H], f32, tag="s2colT")
        s1rows = cpool.tile([1, H * nb], bf16, tag="s1rows")
        x_nat = cpool.tile([128, icn * FCAT], bf16, tag="x_nat")
        xT_sb = cpool.tile([128, kcn * nb], bf16, tag="xT")
        h2g_sb = cpool.tile([128, jcn * (GRW - 1)], bf16, tag="h2g")

        # ---- Phase 1+2: own-block s1 rows, E-broadcasts, h-build, F factors ----
        with tc.tile_pool(name="pp_a", bufs=1, space="PSUM") as pp_a:
            for hh in range(H):
                ps1 = pp_a.tile([1, nb], f32, tag="s1own", name=f"ps1_{hh}")
                for kc in range(kcn):
                    nc.tensor.matmul(ps1[:],
                                     WA8_sb[:, kc * 2 * H + hh: kc * 2 * H + hh + 1],
                                     XTo_sb[:, kc * nb:(kc + 1) * nb],
                                     start=(kc == 0), stop=(kc == kcn - 1))
                nc.scalar.activation(s1rows[:, hh * nb:(hh + 1) * nb], ps1[:],
                                     AF.Identity, bias=cb[0:1, hh:hh + 1])

            # batched separable-exp row factors, then per-head broadcasts
            e1a = spool.tile([1, H * nb], bf16, tag="e1", name="e1a")
            nc.scalar.activation(e1a[:], s1rows[:], AF.Exp)
            e2a = spool.tile([1, H * nb], bf16, tag="e2", name="e2a")
            nc.scalar.activation(e2a[:], s1rows[:], AF.Exp, scale=0.2)
            E1bs, E2bs = [], []
            for h in range(H):
                pe1 = pp_a.tile([128, nb], f32, tag="eb", bufs=2, name=f"pe1_{h}")
                nc.tensor.matmul(pe1[:], onesb[:], e1a[:, h * nb:(h + 1) * nb])
                E1b = cpool.tile([128, nb], bf16, tag=f"E1b_{h}", name=f"E1b_{h}")
                nc.any.tensor_copy(E1b[:], pe1[:])
                pe2 = pp_a.tile([128, nb], f32, tag="eb", bufs=2, name=f"pe2_{h}")
                nc.tensor.matmul(pe2[:], onesb[:], e2a[:, h * nb:(h + 1) * nb])
                E2b = cpool.tile([128, nb], bf16, tag=f"E2b_{h}", name=f"E2b_{h}")
                nc.any.tensor_copy(E2b[:], pe2[:])
                E1bs.append(E1b); E2bs.append(E2b)

            # h-build (replicated) + fused s2 columns
            for jc in range(jcn):
                ph = pp_a.tile([128, FCAT], f32, tag="hb", bufs=2,
                               name=f"ph_{jc}")
                ps = pp_a.tile([128, H], f32, tag="sf", bufs=2, name=f"psf_{jc}")
                for kc in range(kcn):
                    lhs = XT_sb[:, kc * n + jc * 128: kc * n + jc * 128 + 128]
                    nc.tensor.matmul(ph[:], lhs, WC_sb[:, kc * FCAT:(kc + 1) * FCAT],
                                     start=(kc == 0), stop=(kc == kcn - 1))
                    nc.tensor.matmul(ps[:], lhs, WA4_sb[:, kc * H:(kc + 1) * H],
                                     start=(kc == 0), stop=(kc == kcn - 1))
                hv = h_all[:].rearrange("p (h jc w) -> p h jc w", h=H, jc=jcn)
                if jc % 2 == 0:
                    nc.scalar.copy(hv[:, :, jc, 1:HCW],
                                   ph[:].rearrange("p (h f) -> p h f", h=H))
                else:
                    nc.vector.tensor_copy(hv[:, :, jc, 1:HCW],
                                          ph[:].rearrange("p (h f) -> p h f", h=H))
                sv = s2colT[:].rearrange("p (jc h) -> p jc h", jc=jcn)
                nc.any.tensor_copy(sv[:, jc, :], ps[:])

            # F factors, chunked so early score tiles don't wait on all of h-build
            F1s, F2s = [], []
            for h in range(H):
                FCH = 8 if h == 0 else jcn
                sc_h = s2colT[:].rearrange("p (jc hh) -> p jc hh", jc=jcn)[:, :, h]
                F1 = cpool.tile([128, jcn], f32, tag=f"F1_{h}", name=f"F1_{h}")
                F2 = cpool.tile([128, jcn], f32, tag=f"F2_{h}", name=f"F2_{h}")
                for c0 in range(0, jcn, FCH):
                    c1_ = min(jcn, c0 + FCH)
                    nc.scalar.activation(F1[:, c0:c1_], sc_h[:, c0:c1_], AF.Exp)
                    nc.scalar.activation(F2[:, c0:c1_], sc_h[:, c0:c1_], AF.Exp,
                                         scale=0.2)
                F1s.append(F1); F2s.append(F2)

        # ---- Phase 3: per-head layer-1 attention streams ----
        with tc.tile_pool(name="pp_c", bufs=1, space="PSUM") as pp_c:
            for h in range(H):
                E1b, E2b, F1, F2 = E1bs[h], E2bs[h], F1s[h], F2s[h]
                vps = []
                for ic in range(icn):
                    vt_ = pp_c.tile([128, HCW], f32, tag=f"vl{ic}",
                                    name=f"vl_{h}_{ic}")
                    vps.append(vt_)
                for jc in range(jcn):
                    A = work.tile([128, nb], bf16, tag="a")
                    nc.scalar.activation(A[:], E1b[:], AF.Identity,
                                         scale=F1[:, jc:jc + 1])
                    B = work.tile([128, nb], bf16, tag="b")
                    if jc % 16 == 11:
                        nc.vector.tensor_scalar_mul(B[:], E2b[:], F2[:, jc:jc + 1])
                    else:
                        nc.gpsimd.tensor_scalar_mul(B[:], E2b[:], F2[:, jc:jc + 1])
                    mx = work.tile([128, nb], bf16, tag="c")
                    nc.vector.tensor_tensor(mx[:], A[:], B[:], OP.max)
                    p = work.tile([128, nb], bf16, tag="p", bufs=14)
                    nc.vector.tensor_tensor(p[:], mx[:],
                                            MT_sb[:, jc * nb:(jc + 1) * nb], OP.mult)
                    base = (h * jcn + jc) * HCW
                    for ic in range(icn):
                        iw = min(128, nb - ic * 128)
                        nc.tensor.matmul(vps[ic][0:iw, :],
                                         p[:, ic * 128: ic * 128 + iw],
                                         h_all[:, base:base + HCW],
                                         start=(jc == 0), stop=(jc == jcn - 1))

                # finalize: normalize by denominator col, +b_h, elu(+1)
                for ic in range(icn):
                    iw = min(128, nb - ic * 128)
                    rc = spool.tile([128, 1], f32, tag="rc")
                    nc.vector.reciprocal(rc[0:iw, :], vps[ic][0:iw, 0:1])
                    vn = spool.tile([128, FH], bf16, tag="vn")
                    nc.vector.tensor_scalar_mul(vn[0:iw, :], vps[ic][0:iw, 1:HCW],
                                                rc[0:iw, :])
                    vb = spool.tile([128, FH], bf16, tag="vb")
                    nc.vector.tensor_tensor(vb[0:iw, :], vn[0:iw, :],
                                            BHB_sb[0:iw, h * FH:(h + 1) * FH],
                                            OP.add)
                    r = spool.tile([128, FH], bf16, tag="r")
                    nc.vector.tensor_scalar_max(r[0:iw, :], vb[0:iw, :], 0.0)
                    mn = spool.tile([128, FH], bf16, tag="mn")
                    nc.vector.tensor_scalar_min(mn[0:iw, :], vb[0:iw, :], 0.0)
                    E = spool.tile([128, FH], bf16, tag="E")
                    nc.scalar.activation(E[0:iw, :], mn[0:iw, :], AF.Exp)
                    nc.vector.tensor_tensor(
                        x_nat[0:iw, ic * FCAT + h * FH: ic * FCAT + (h + 1) * FH],
                        r[0:iw, :], E[0:iw, :], OP.add)
                    # transpose this head's x block to xT right away via the
                    # DMA xbar (idle DMA engines; frees VectorE/ScalarE evacs)
                    nc.sync.dma_start_transpose(
                        xT_sb[:, h * nb + ic * 128: h * nb + ic * 128 + iw],
                        x_nat[0:iw,
                              ic * FCAT + h * 128: ic * FCAT + h * 128 + 128])

        # ---- Phase 4: gather input = [x_raw . wao2 | 0 | x_raw @ Wo] ----
        GRH = GRW - 1  # 65: [ones-slot | h2 features]
        gin_s = dpool.tile([nb, 1], bf16, tag="gin_s")
        gout_s = dpool.tile([n, 1], bf16, tag="gout_s")
        gin = dpool.tile([nb, GRH], bf16, tag="gin")
        gout = dpool.tile([n, GRH], bf16, tag="gout")
        with tc.tile_pool(name="pp_d", bufs=1, space="PSUM") as pp_d:
            for ic in range(icn):
                iw = min(128, nb - ic * 128)
                pg = pp_d.tile([128, GRW], f32, tag="g", name=f"pg_{ic}")
                for kc in range(kcn):
                    nc.tensor.matmul(
                        pg[0:iw, :],
                        xT_sb[:, kc * nb + ic * 128: kc * nb + ic * 128 + iw],
                        GR_sb[:, kc * GRW:(kc + 1) * GRW],
                        start=(kc == 0), stop=(kc == kcn - 1))
                gsb = spool.tile([128, GRW], bf16, tag="gsb")
                nc.any.tensor_copy(gsb[0:iw, :], pg[0:iw, :])
                nc.sync.dma_start(gin_s[ic * 128: ic * 128 + iw, :],
                                  gsb[0:iw, 0:1])
                nc.sync.dma_start(gin[ic * 128: ic * 128 + iw, :],
                                  gsb[0:iw, 1:GRW])

            # own-block s1_o row + separable-exp factors for layer 2
            ps1o = pp_d.tile([1, nb], f32, tag="s1o")
            for kc in range(kcn):
                nc.tensor.matmul(ps1o[:], WAO1_sb[:, kc:kc + 1],
                                 xT_sb[:, kc * nb:(kc + 1) * nb],
                                 start=(kc == 0), stop=(kc == kcn - 1))
            s1orow = spool.tile([1, nb], f32, tag="s1orow")
            nc.scalar.copy(s1orow[:], ps1o[:])
            e1o = spool.tile([1, nb], bf16, tag="e1", name="e1o")
            nc.scalar.activation(e1o[:], s1orow[:], AF.Exp,
                                 bias=cb[0:1, 2 * H:2 * H + 1])
            e2o = spool.tile([1, nb], bf16, tag="e2", name="e2o")
            nc.scalar.activation(e2o[:], s1orow[:], AF.Exp, scale=0.2,
                                 bias=cb[0:1, 2 * H + 1:2 * H + 2])
            pe1o = pp_d.tile([128, nb], f32, tag="ebo", bufs=2, name="pe1o")
            nc.tensor.matmul(pe1o[:], onesb[:], e1o[:])
            E1bo = spool.tile([128, nb], bf16, tag="E1b", name="E1bo")
            nc.any.tensor_copy(E1bo[:], pe1o[:])
            pe2o = pp_d.tile([128, nb], f32, tag="ebo", bufs=2, name="pe2o")
            nc.tensor.matmul(pe2o[:], onesb[:], e2o[:])
            E2bo = spool.tile([128, nb], bf16, tag="E2b", name="E2bo")
            nc.any.tensor_copy(E2bo[:], pe2o[:])

            # ---- AllGather: tiny s2_o first, then h2 features; the L2
            # score stream only needs s2_o, so it overlaps the big gather ----
            if skip_collective:
                for cc_ in range(ncores):
                    nc.gpsimd.dma_start(gout_s[cc_ * nb:(cc_ + 1) * nb, :],
                                        gin_s[:])
                for cc_ in range(ncores):
                    nc.gpsimd.dma_start(gout[cc_ * nb:(cc_ + 1) * nb, :], gin[:])
            else:
                nc.gpsimd.collective_compute(
                    "AllGather", OP.bypass,
                    replica_groups=[list(range(ncores))],
                    ins=[gin_s[:].opt()], outs=[gout_s[:].opt()],
                )
                nc.gpsimd.collective_compute(
                    "AllGather", OP.bypass,
                    replica_groups=[list(range(ncores))],
                    ins=[gin[:].opt()], outs=[gout[:].opt()],
                )
            s2og = spool.tile([128, jcn], bf16, tag="s2og")
            nc.sync.dma_start(s2og[:].rearrange("p (jc one) -> p jc one", jc=jcn),
                              gout_s[:].rearrange("(jc p) one -> p jc one", p=128))
            F1o = spool.tile([128, jcn], f32, tag="F1", name="F1o")
            nc.scalar.activation(F1o[:], s2og[:], AF.Exp)
            F2o = spool.tile([128, jcn], f32, tag="F2", name="F2o")
            nc.scalar.activation(F2o[:], s2og[:], AF.Exp, scale=0.2)
            nc.sync.dma_start(h2g_sb[:].rearrange("p (jc x) -> p jc x", jc=jcn),
                              gout[:].rearrange("(jc p) x -> p jc x", p=128))
            nc.vector.memset(h2g_sb[:, 0:jcn * GRH:GRH], 1.0)  # ones col

        # ---- Phase 5: layer-2 attention (natural-out aggregation) ----
        with tc.tile_pool(name="pp_e", bufs=1, space="PSUM") as pp_e:
            vps = []
            for ic in range(icn):
                vt_ = pp_e.tile([128, 1 + FO], f32, tag=f"v{ic}", name=f"v2_{ic}")
                vps.append(vt_)
            for jc in range(jcn):
                A = work.tile([128, nb], bf16, tag="a")
                nc.scalar.activation(A[:], E1bo[:], AF.Identity,
                                     scale=F1o[:, jc:jc + 1])
                B = work.tile([128, nb], bf16, tag="b")
                if jc % 16 == 11:
                    nc.vector.tensor_scalar_mul(B[:], E2bo[:], F2o[:, jc:jc + 1])
                else:
                    nc.gpsimd.tensor_scalar_mul(B[:], E2bo[:], F2o[:, jc:jc + 1])
                mx = work.tile([128, nb], bf16, tag="c")
                nc.vector.tensor_tensor(mx[:], A[:], B[:], OP.max)
                p2 = work.tile([128, nb], bf16, tag="p", bufs=14)
                nc.vector.tensor_tensor(p2[:], mx[:],
                                        MT_sb[:, jc * nb:(jc + 1) * nb], OP.mult)
                for ic in range(icn):
                    iw = min(128, nb - ic * 128)
                    nc.tensor.matmul(
                        vps[ic][0:iw, :], p2[:, ic * 128: ic * 128 + iw],
                        h2g_sb[:, jc * GRH:(jc + 1) * GRH],
                        start=(jc == 0), stop=(jc == jcn - 1))

            # ---- Phase 6: normalize, +beta, elu(+1), log_softmax, store ----
            # wave emission: each op across all ic blocks, chains interleave
            iws = [min(128, nb - ic * 128) for ic in range(icn)]
            rcs = [spool.tile([128, 1], f32, tag="rc6", bufs=icn, name=f"rc6_{ic}")
                   for ic in range(icn)]
            for ic in range(icn):
                nc.vector.reciprocal(rcs[ic][0:iws[ic], :], vps[ic][0:iws[ic], 0:1])
            vvs = [spool.tile([128, FO], f32, tag="vv", bufs=icn, name=f"vv_{ic}")
                   for ic in range(icn)]
            for ic in range(icn):
                nc.vector.tensor_scalar_mul(vvs[ic][0:iws[ic], :],
                                            vps[ic][0:iws[ic], 1:1 + FO],
                                            rcs[ic][0:iws[ic], :])
            vts = [spool.tile([128, FO], f32, tag="vt", bufs=icn, name=f"vt_{ic}")
                   for ic in range(icn)]
            for ic in range(icn):
                nc.vector.tensor_tensor(vts[ic][0:iws[ic], :], vvs[ic][0:iws[ic], :],
                                        BETA_sb[0:iws[ic], :], OP.add)
            rs = [spool.tile([128, FO], f32, tag="r2", bufs=icn, name=f"r2_{ic}")
                  for ic in range(icn)]
            mns = [spool.tile([128, FO], f32, tag="mn2", bufs=icn, name=f"mn2_{ic}")
                   for ic in range(icn)]
            Es = [spool.tile([128, FO], f32, tag="E2", bufs=icn, name=f"Eo_{ic}")
                  for ic in range(icn)]
            for ic in range(icn):
                nc.vector.tensor_scalar_max(rs[ic][0:iws[ic], :],
                                            vts[ic][0:iws[ic], :], 0.0)
                nc.vector.tensor_scalar_min(mns[ic][0:iws[ic], :],
                                            vts[ic][0:iws[ic], :], 0.0)
            for ic in range(icn):
                nc.scalar.activation(Es[ic][0:iws[ic], :], mns[ic][0:iws[ic], :],
                                     AF.Exp)
            us, nms, ses = [], [], []
            for ic in range(icn):
                u = spool.tile([128, FO], f32, tag="u", bufs=icn, name=f"u_{ic}")
                nc.vector.tensor_tensor(u[0:iws[ic], :], rs[ic][0:iws[ic], :],
                                        Es[ic][0:iws[ic], :], OP.add)
                us.append(u)
            for ic in range(icn):
                nm = spool.tile([128, 1], f32, tag="nm", bufs=icn, name=f"nm_{ic}")
                nc.vector.tensor_reduce(nm[0:iws[ic], :], us[ic][0:iws[ic], :],
                                        AX.X, OP.max, negate=True)
                nms.append(nm)
            for ic in range(icn):
                eu = spool.tile([128, FO], f32, tag="eu")
                se = spool.tile([128, 1], f32, tag="se", bufs=icn, name=f"se_{ic}")
                nc.scalar.activation(eu[0:iws[ic], :], us[ic][0:iws[ic], :],
                                     AF.Exp, bias=nms[ic][0:iws[ic], :],
                                     accum_out=se[0:iws[ic], :])
                ses.append(se)
            # group the Lns after all Exps (one activation-table switch)
            for ic in range(icn):
                iw = min(128, nb - ic * 128)
                L = spool.tile([128, 1], f32, tag="L", name=f"L_{ic}")
                nc.scalar.activation(L[0:iw, :], ses[ic][0:iw, :], AF.Ln)
                cc2 = spool.tile([128, 1], f32, tag="cc2", name=f"cc2_{ic}")
                nc.vector.tensor_tensor(cc2[0:iw, :], nms[ic][0:iw, :],
                                        L[0:iw, :], OP.subtract)
                outf = spool.tile([128, FO], f32, tag="outf", name=f"outf_{ic}")
                nc.vector.tensor_scalar_add(outf[0:iw, :], us[ic][0:iw, :],
                                            cc2[0:iw, :])
                nc.sync.dma_start(d_OUT.ap()[ic * 128: ic * 128 + iw, :],
                                  outf[0:iw, :])

    return d_OUT


def run_gat(inputs, cfg=None, trace=False):
    import concourse.bacc as bacc
    import concourse.tile as tile
    from concourse.bass_utils import run_bass_kernel_spmd

    cfg = cfg or _cfg()
    in_maps, consts = host_prep(cfg=cfg, **inputs)

    nc = bacc.Bacc("TRN2", target_bir_lowering=False, debug=False,
                   num_devices=cfg["n_cores"])
    with tile.TileContext(nc) as tc:
        build_program(nc, tc, cfg, consts)
    nc.compile()

    res = run_bass_kernel_spmd(nc, in_maps, list(range(cfg["n_cores"])),
                               trace=trace)
    out = np.concatenate([res.results[c]["OUT"] for c in range(cfg["n_cores"])],
                         axis=0)
    return out.astype(np.float32), res


def kernel(**inputs) -> np.ndarray:
    out, _ = run_gat(inputs)
    return out



# revision 34
# speedup vs baseline: 61.0694x; 61.0694x over previous
"""GAT (2-layer, 4-head) Trainium2 Bass kernel, SPMD over 8 NeuronCores.

Sharding: 1D row partition of N. Each core computes its N/8-row block of
both attention layers. h (per-head projected features) is computed
replicated on every core (cheap); layer-2 features are exchanged with a
single merged AllGather.

Key math (per head, with t = s1_i + s2_j + c the raw attention logit):
  exp(leaky_relu(t)) = E1_i * F2_j * max(G_j, r_i)
    where E1 = exp(s1+c), F2 = exp(0.2*s2), G = exp(0.8*s2),
          r = exp(-0.8*(s1+c)).
  E1_i cancels in the softmax normalization, so it is never computed.
  F2_j is folded into the aggregation feature matrix (including the
  denominator "ones" slot, which becomes F2 itself). The entire masked
  score tile then needs ONE vector op per (head, j-chunk):
      q = (r_bcast max G_col) * mask        [scalar_tensor_tensor]
  and the aggregation matmul contracts q against [F2 | F2*h].

- Scores are computed in TRANSPOSED form q[j, i] (source node j on
  partitions, this core's destination rows i on the free axis), so the
  softmax contraction axis is the partition axis; no N x N transposes
  are ever needed on device (X and the mask are fed pre-transposed).
- Softmax denominators come free from the aggregation matmul via the
  F2 column of the feature matrix (one reciprocal per output row).
- The layer-2 exchange gathers [G_o | F2o | F2o*(x@Wo)] per row: the
  receiver uses the G column directly as the ts-max scalar and the
  remaining 65 columns directly as the aggregation RHS. One AllGather.
- elu's -1, all biases, and the attention-vector projections are folded
  on the host into score constants / a post-normalize bias add.
"""

import numpy as np
import ml_dtypes
from contextlib import ExitStack

BF16 = ml_dtypes.bfloat16
FP8 = ml_dtypes.float8_e4m3

# problem constants (hardcoded per contract)
N, FIN, FH, H, FO = 4096, 512, 128, 4, 64
N_CORES = 8
FCAT = H * FH  # 512
HCW = FH + 1   # 129: [F2-slot | 128 features] per (head, jc)
GRW = 2 + FO   # gather row width: [G_o | F2o | F2o * h2 features]
GRH = 1 + FO   # L2 aggregation rhs width: [F2o | F2o*h2]


def _cfg(n=N, n_cores=N_CORES):
    nb = n // n_cores        # rows per core
    return dict(
        n=n, nb=nb, n_cores=n_cores,
        jc=n // 128,          # j (source-node) chunks of 128
        kc=FIN // 128,        # contraction chunks for X @ W
        ic=(nb + 127) // 128, # i chunks for natural-out aggregation
    )


def host_prep(X, adjacency_matrix, W_h, b_h, a1_h, a2_h, ab_h,
              W_o, b_o, a1_o, a2_o, ab_o, cfg):
    """Fold weights / biases on the host; produce per-core input maps."""
    f32 = np.float32
    X = np.asarray(X, f32)
    adj = np.asarray(adjacency_matrix)
    W_h, b_h = np.asarray(W_h, f32), np.asarray(b_h, f32)
    a1_h, a2_h, ab_h = np.asarray(a1_h, f32), np.asarray(a2_h, f32), np.asarray(ab_h, f32)
    W_o, b_o = np.asarray(W_o, f32), np.asarray(b_o, f32)
    a1_o, a2_o, ab_o = np.asarray(a1_o, f32), np.asarray(a2_o, f32), np.asarray(ab_o, f32)

    nb, ncores = cfg["nb"], cfg["n_cores"]
    n = cfg["n"]
    jcn, kcn = cfg["jc"], cfg["kc"]

    def tile_k(a):
        # [FIN, W] -> [128, kcn*W] in kc-major SBUF layout (pre-tiled on host
        # so the device DMA is a single contiguous row-block copy)
        W = a.shape[1]
        return np.ascontiguousarray(
            a.reshape(kcn, 128, W).transpose(1, 0, 2).reshape(128, kcn * W))

    def tile_k2(a):
        # [FIN, W] -> [128, (kcn//2)*2*W] kc-PAIR layout for DoubleRow rhs
        W = a.shape[1]
        return np.ascontiguousarray(
            a.reshape(kcn // 2, 2, 128, W).transpose(2, 0, 1, 3)
            .reshape(128, kcn * W))

    XT = np.ascontiguousarray(X.T).astype(BF16)                  # [FIN, n]
    maskT = np.ascontiguousarray((adj > 0).T.astype(f32)).astype(BF16)
    # XT in jc-major pre-tiled layout: [128, jcn, kcn, 128]
    XTt = np.ascontiguousarray(
        XT.reshape(kcn, 128, jcn, 128).transpose(1, 2, 0, 3).reshape(128, -1))
    # MT pre-tiled: [128, jcn*nb] per core (sliced below)
    MTt_full = maskT.reshape(jcn, 128, n).transpose(1, 0, 2)     # [128,jcn,n]

    wa1 = np.stack([W_h[h] @ a1_h[h] for h in range(H)], axis=1)  # [FIN, H]
    wa2 = np.stack([W_h[h] @ a2_h[h] for h in range(H)], axis=1)  # [FIN, H]
    WC = tile_k(np.concatenate([W_h[h] for h in range(H)], axis=1).astype(BF16))
    WA4 = tile_k(wa2.astype(BF16))
    WA8 = tile_k(np.concatenate([wa1, wa2], axis=1).astype(BF16))
    c1 = np.array([b_h[h] @ a1_h[h] + ab_h[h] for h in range(H)], f32)
    c2 = np.array([b_h[h] @ a2_h[h] for h in range(H)], f32)
    c12 = (c1 + c2).astype(f32)   # layer-1 per-head score constant

    wao1 = W_o @ a1_o
    wao2 = W_o @ a2_o
    # elu(-1) fold: device computes x_raw = elu(z)+1; corrections:
    cS1o = float(b_o @ a1_o + ab_o - wao1.sum())   # s1_o constant
    c2o = float(b_o @ a2_o - wao2.sum())           # s2_o constant
    beta = (b_o - W_o.sum(axis=0)).astype(f32)                    # [FO]
    BETA = np.ascontiguousarray(np.broadcast_to(beta, (128, FO)))

    GR = tile_k(np.concatenate([wao2[:, None], W_o], axis=1).astype(BF16))
    WAO1 = tile_k(wao1[:, None].astype(BF16))
    # b_h broadcast to all partitions, concatenated along heads
    BHB = np.ascontiguousarray(
        np.broadcast_to(np.concatenate([b_h[h] for h in range(H)]),
                        (128, FCAT))).astype(BF16)                # [128, H*FH]

    # score-constant bias columns, broadcast to all partitions:
    # [ -0.8*c12_h (x4) | 0.8*c2o | 0.2*c2o | -0.8*cS1o ]
    cbrow = np.concatenate([-0.8 * c12,
                            [0.8 * c2o, 0.2 * c2o, -0.8 * cS1o]]).astype(f32)
    CB = np.ascontiguousarray(np.broadcast_to(cbrow, (128, H + 3)))

    shared = {
        "XT": XTt, "WC": WC, "WA4": WA4, "WA8": WA8,
        "GR": GR, "WAO1": WAO1, "BHB": BHB, "BETA": BETA, "CB": CB,
    }
    in_maps = []
    for c in range(ncores):
        m = dict(shared)
        m["XTo"] = tile_k(np.ascontiguousarray(XT[:, c * nb:(c + 1) * nb]))
        m["MT"] = np.ascontiguousarray(
            MTt_full[:, :, c * nb:(c + 1) * nb].reshape(128, jcn * nb))
        in_maps.append(m)
    return in_maps, {"c12": c12, "cS1o": cS1o, "c2o": c2o}


def make_dram_io(nc, cfg):
    from concourse import mybir
    f32 = mybir.dt.float32
    bf16 = mybir.dt.bfloat16
    fp8 = mybir.dt.float8e4
    n, nb = cfg["n"], cfg["nb"]
    kcn, jcn = cfg["kc"], cfg["jc"]
    return dict(
        XT=nc.dram_tensor("XT", [128, jcn * kcn * 128], bf16,
                          kind="ExternalInput"),
        XTo=nc.dram_tensor("XTo", [128, kcn * nb], bf16, kind="ExternalInput"),
        MT=nc.dram_tensor("MT", [128, jcn * nb], bf16, kind="ExternalInput"),
        WC=nc.dram_tensor("WC", [128, kcn * FCAT], bf16, kind="ExternalInput"),
        WA4=nc.dram_tensor("WA4", [128, kcn * H], bf16, kind="ExternalInput"),
        WA8=nc.dram_tensor("WA8", [128, kcn * 2 * H], bf16,
                           kind="ExternalInput"),
        GR=nc.dram_tensor("GR", [128, kcn * GRH], bf16, kind="ExternalInput"),
        WAO1=nc.dram_tensor("WAO1", [128, kcn], bf16, kind="ExternalInput"),
        BHB=nc.dram_tensor("BHB", [128, FCAT], bf16, kind="ExternalInput"),
        BETA=nc.dram_tensor("BETA", [128, FO], f32, kind="ExternalInput"),
        CB=nc.dram_tensor("CB", [128, H + 3], f32, kind="ExternalInput"),
    )


def build_program(nc, tc, cfg, consts, skip_collective=False, sfx="", dio=None):
    """Emit the SPMD GAT program into TileContext tc."""
    from concourse import mybir

    f32 = mybir.dt.float32
    bf16 = mybir.dt.bfloat16
    AF = mybir.ActivationFunctionType
    OP = mybir.AluOpType
    AX = mybir.AxisListType

    n, nb, jcn, kcn, icn = cfg["n"], cfg["nb"], cfg["jc"], cfg["kc"], cfg["ic"]
    ncores = cfg["n_cores"]

    if dio is None:
        dio = make_dram_io(nc, cfg)
    d_OUT = nc.dram_tensor("OUT" + sfx, [nb, FO], f32, kind="ExternalOutput")

    ctx = ExitStack()
    with ctx:
        cpool = ctx.enter_context(tc.tile_pool(name="const" + sfx, bufs=1))
        work = ctx.enter_context(tc.tile_pool(name="work" + sfx, bufs=6))
        spool = ctx.enter_context(tc.tile_pool(name="small" + sfx, bufs=2))
        dpool = ctx.enter_context(
            tc.tile_pool(name="dram" + sfx, bufs=1, space="DRAM"))

        def load(name, dram, parts, width, dt=bf16, eng=None):
            t = cpool.tile([parts, width], dt, tag=name, name=f"ld_{name}")
            (eng or nc.sync).dma_start(t[:], dram.ap())
            return t

        # all inputs are host-pre-tiled to their SBUF layout: every DMA is a
        # contiguous row-block copy (>=512B runs), split in jc chunks so the
        # h-build / score streams can start as soon as their chunk lands.
        # MT goes through the Activation HWDGE queue, the rest through SP.
        XTo_sb = load("XTo", dio["XTo"], 128, kcn * nb)
        cb = load("CB", dio["CB"], 128, H + 3, dt=f32)
        WA8_sb = load("WA8", dio["WA8"], 128, kcn * 2 * H)
        WC_sb = load("WC", dio["WC"], 128, kcn * FCAT)
        WA4_sb = load("WA4", dio["WA4"], 128, kcn * H)
        XT_sb = cpool.tile([128, jcn * kcn * 128], bf16, tag="XT", name="ld_XT")
        MT_sb = cpool.tile([128, jcn * nb], bf16, tag="MT", name="ld_MT")
        JG = 4  # jc chunks per DMA
        XW, MW = kcn * 128, nb
        for g0 in range(0, jcn, JG):
            nc.sync.dma_start(XT_sb[:, g0 * XW:(g0 + JG) * XW],
                              dio["XT"].ap()[:, g0 * XW:(g0 + JG) * XW])
            nc.scalar.dma_start(MT_sb[:, g0 * MW:(g0 + JG) * MW],
                                dio["MT"].ap()[:, g0 * MW:(g0 + JG) * MW])
        GR_sb = load("GR", dio["GR"], 128, kcn * GRH)
        WAO1_sb = load("WAO1", dio["WAO1"], 128, kcn)
        BHB_sb = load("BHB", dio["BHB"], 128, FCAT)
        BETA_sb = load("BETA", dio["BETA"], 128, FO, dt=f32)

        onesb = cpool.tile([1, 128], bf16, tag="ones")
        nc.vector.memset(onesb[:], 1.0)

        # h_all: per (head, jc): [F2 slot | 128 F2-scaled features]
        h_all = cpool.tile([128, H * jcn * HCW], bf16, tag="h_all")
        G_all = cpool.tile([128, jcn * H], f32, tag="G_all")   # exp(0.8 s2)
        F_all = cpool.tile([128, jcn * H], f32, tag="F_all")   # exp(0.2 s2)
        Rbs = [cpool.tile([128, nb], bf16, tag=f"Rb_{h}", name=f"Rb_{h}")
               for h in range(H)]
        x_nat = cpool.tile([128, icn * FCAT], bf16, tag="x_nat")
        xT_sb = cpool.tile([128, kcn * nb], bf16, tag="xT")
        h2g_sb = cpool.tile([128, jcn * GRW], bf16, tag="h2g")
        Go_all = cpool.tile([128, jcn], f32, tag="Go_all")

        # ---- Phase 1: own-block r rows + broadcasts (own PSUM scope) ----
        with tc.tile_pool(name="pp_r" + sfx, bufs=1, space="PSUM") as pp_r:
            for hh in range(H):
                ps1 = pp_r.tile([1, nb], f32, tag="s1own", bufs=2,
                                name=f"ps1_{hh}")
                for kc in range(kcn):
                    nc.tensor.matmul(ps1[:],
                                     WA8_sb[:, kc * 2 * H + hh: kc * 2 * H + hh + 1],
                                     XTo_sb[:, kc * nb:(kc + 1) * nb],
                                     start=(kc == 0), stop=(kc == kcn - 1))
                rr = spool.tile([1, nb], bf16, tag="rr", bufs=2, name=f"rr_{hh}")
                nc.scalar.activation(rr[:], ps1[:], AF.Exp,
                                     scale=-0.8, bias=cb[0:1, hh:hh + 1])
                pRb = pp_r.tile([128, nb], f32, tag="pRb", bufs=2,
                                name=f"pRb_{hh}")
                nc.tensor.matmul(pRb[:], onesb[:], rr[:])
                if hh % 2 == 0:
                    nc.vector.tensor_copy(Rbs[hh][:], pRb[:])
                else:
                    nc.scalar.copy(Rbs[hh][:], pRb[:])

        # ---- Phases 2+3 share PSUM via sibling pools so the h-build and
        # the attention streams pipeline jc-by-jc (no inter-pool barrier) ----
        with tc.tile_pool(name="pp_b" + sfx, bufs=1, space="PSUM") as pp_b, \
             tc.tile_pool(name="pp_c" + sfx, bufs=1, space="PSUM") as pp_c:
            # h-build (replicated) + fused F2/G score factors
            hv = h_all[:].rearrange("p (h jc w) -> p h jc w", h=H, jc=jcn)
            gv = G_all[:].rearrange("p (jc h) -> p jc h", jc=jcn)
            fv = F_all[:].rearrange("p (jc h) -> p jc h", jc=jcn)
            for jc in range(jcn):
                ph = pp_b.tile([128, FCAT], f32, tag="hb", bufs=2,
                               name=f"ph_{jc}")
                psf = pp_b.tile([128, H], f32, tag="sf", name=f"psf_{jc}")
                for kc in range(kcn):
                    base = (jc * kcn + kc) * 128
                    lhs = XT_sb[:, base: base + 128]
                    nc.tensor.matmul(ph[:], lhs, WC_sb[:, kc * FCAT:(kc + 1) * FCAT],
                                     start=(kc == 0), stop=(kc == kcn - 1))
                    nc.tensor.matmul(psf[:], lhs, WA4_sb[:, kc * H:(kc + 1) * H],
                                     start=(kc == 0), stop=(kc == kcn - 1))
                nc.scalar.activation(fv[:, jc, :], psf[:], AF.Exp, scale=0.2)
                nc.scalar.activation(gv[:, jc, :], psf[:], AF.Exp, scale=0.8)
                # F2 slots (bf16 copy of F_all) + F2-scaled features
                nc.vector.tensor_copy(hv[:, :, jc, 0:1], fv[:, jc, :])
                for h in range(H):
                    fcol = fv[:, jc, h:h + 1]
                    src = ph[:, h * FH:(h + 1) * FH]
                    dst = hv[:, h, jc, 1:HCW]
                    e = (jc * H + h) % 2
                    if e == 0:
                        nc.scalar.activation(dst, src, AF.Identity, scale=fcol)
                    else:
                        nc.vector.tensor_scalar_mul(dst, src, fcol)

            # ---- Phase 3: per-head layer-1 attention streams ----
            for h in range(H):
                vps = []
                for ic in range(icn):
                    vt_ = pp_c.tile([128, HCW], f32, tag=f"vl{ic}",
                                    name=f"vl_{h}_{ic}")
                    vps.append(vt_)
                for jc in range(jcn):
                    q = work.tile([128, nb], bf16, tag="p", bufs=14)
                    idx = h * jcn + jc
                    Gcol = G_all[:, jc * H + h: jc * H + h + 1]
                    Mt = MT_sb[:, jc * nb:(jc + 1) * nb]
                    mx = work.tile([128, nb], bf16, tag="c")
                    if idx % 11 == 10:
                        # ts_max in DVE 4x mode (~150ns), keeps DVE/Pool even
                        nc.vector.tensor_scalar_max(mx[:], Rbs[h][:], Gcol)
                    else:
                        nc.gpsimd.tensor_scalar_max(mx[:], Rbs[h][:], Gcol)
                    nc.vector.tensor_tensor(q[:], mx[:], Mt, OP.mult)
                    base = (h * jcn + jc) * HCW
                    for ic in range(icn):
                        iw = min(128, nb - ic * 128)
                        nc.tensor.matmul(vps[ic][0:iw, :],
                                         q[:, ic * 128: ic * 128 + iw],
                                         h_all[:, base:base + HCW],
                                         start=(jc == 0), stop=(jc == jcn - 1))

                # finalize: normalize by F2-denominator col, +b_h, elu(+1)
                for ic in range(icn):
                    iw = min(128, nb - ic * 128)
                    rc = spool.tile([128, 1], f32, tag="rc")
                    nc.vector.reciprocal(rc[0:iw, :], vps[ic][0:iw, 0:1])
                    vn = spool.tile([128, FH], bf16, tag="vn")
                    nc.scalar.activation(vn[0:iw, :], vps[ic][0:iw, 1:HCW],
                                         AF.Identity, scale=rc[0:iw, :])
                    vb = spool.tile([128, FH], bf16, tag="vb")
                    nc.gpsimd.tensor_tensor(vb[0:iw, :], vn[0:iw, :],
                                            BHB_sb[0:iw, h * FH:(h + 1) * FH],
                                            OP.add)
                    r = spool.tile([128, FH], bf16, tag="r")
                    nc.gpsimd.tensor_scalar_max(r[0:iw, :], vb[0:iw, :], 0.0)
                    mn = spool.tile([128, FH], bf16, tag="mn")
                    nc.vector.tensor_scalar_min(mn[0:iw, :], vb[0:iw, :], 0.0)
                    E = spool.tile([128, FH], bf16, tag="E")
                    nc.scalar.activation(E[0:iw, :], mn[0:iw, :], AF.Exp)
                    nc.vector.tensor_tensor(
                        x_nat[0:iw, ic * FCAT + h * FH: ic * FCAT + (h + 1) * FH],
                        r[0:iw, :], E[0:iw, :], OP.add)
                    # transpose this head's x block to xT right away via the
                    # DMA xbar (idle DMA engines; frees VectorE/ScalarE evacs)
                    nc.sync.dma_start_transpose(
                        xT_sb[:, h * nb + ic * 128: h * nb + ic * 128 + iw],
                        x_nat[0:iw,
                              ic * FCAT + h * 128: ic * FCAT + h * 128 + 128])

        # ---- Phase 4: gather input = [G_o | F2o | F2o * (x_raw @ Wo)] ----
        gin = dpool.tile([nb, GRW], bf16, tag="gin")
        gout = dpool.tile([n, GRW], bf16, tag="gout")
        with tc.tile_pool(name="pp_d" + sfx, bufs=1, space="PSUM") as pp_d:
            for ic in range(icn):
                iw = min(128, nb - ic * 128)
                pg = pp_d.tile([128, GRH], f32, tag="g", name=f"pg_{ic}")
                for kc in range(kcn):
                    nc.tensor.matmul(
                        pg[0:iw, :],
                        xT_sb[:, kc * nb + ic * 128: kc * nb + ic * 128 + iw],
                        GR_sb[:, kc * GRH:(kc + 1) * GRH],
                        start=(kc == 0), stop=(kc == kcn - 1))
                gsb = spool.tile([128, GRW], bf16, tag="gsb")
                gf2 = spool.tile([128, 1], f32, tag="gf2")
                nc.scalar.activation(gsb[0:iw, 0:1], pg[0:iw, 0:1], AF.Exp,
                                     scale=0.8, bias=cb[0:iw, H:H + 1])
                nc.scalar.activation(gf2[0:iw, :], pg[0:iw, 0:1], AF.Exp,
                                     scale=0.2, bias=cb[0:iw, H + 1:H + 2])
                nc.vector.tensor_copy(gsb[0:iw, 1:2], gf2[0:iw, :])
                nc.vector.tensor_scalar_mul(gsb[0:iw, 2:GRW], pg[0:iw, 1:GRH],
                                            gf2[0:iw, :])
                nc.sync.dma_start(gin[ic * 128: ic * 128 + iw, :], gsb[0:iw, :])

            # own-block s1_o row -> r_o broadcast for layer 2
            ps1o = pp_d.tile([1, nb], f32, tag="s1o")
            for kc in range(kcn):
                nc.tensor.matmul(ps1o[:], WAO1_sb[:, kc:kc + 1],
                                 xT_sb[:, kc * nb:(kc + 1) * nb],
                                 start=(kc == 0), stop=(kc == kcn - 1))
            rro = spool.tile([1, nb], bf16, tag="rr", name="rro")
            nc.scalar.activation(rro[:], ps1o[:], AF.Exp, scale=-0.8,
                                 bias=cb[0:1, H + 2:H + 3])
            pRbo = pp_d.tile([128, nb], f32, tag="pRbo", name="pRbo")
            nc.tensor.matmul(pRbo[:], onesb[:], rro[:])
            Rbo = spool.tile([128, nb], bf16, tag="Rbo", name="Rbo")
            nc.vector.tensor_copy(Rbo[:], pRbo[:])

            # ---- single merged AllGather ----
            if skip_collective:
                for cc_ in range(ncores):
                    nc.gpsimd.dma_start(gout[cc_ * nb:(cc_ + 1) * nb, :], gin[:])
            else:
                nc.gpsimd.collective_compute(
                    "AllGather", OP.bypass,
                    replica_groups=[list(range(ncores))],
                    ins=[gin[:].opt()], outs=[gout[:].opt()],
                )
            nc.sync.dma_start(h2g_sb[:].rearrange("p (jc x) -> p jc x", jc=jcn),
                              gout[:].rearrange("(jc p) x -> p jc x", p=128))
            # f32 copy of the G_o columns (ts-max scalar operand must be f32)
            nc.scalar.copy(
                Go_all[:].rearrange("p (jc one) -> p jc one", jc=jcn),
                h2g_sb[:].rearrange("p (jc x) -> p jc x", jc=jcn)[:, :, 0:1])

        # ---- Phase 5: layer-2 attention (natural-out aggregation) ----
        with tc.tile_pool(name="pp_e" + sfx, bufs=1, space="PSUM") as pp_e:
            vps = []
            for ic in range(icn):
                vt_ = pp_e.tile([128, GRH], f32, tag=f"v{ic}", name=f"v2_{ic}")
                vps.append(vt_)
            for jc in range(jcn):
                q2 = work.tile([128, nb], bf16, tag="p", bufs=14)
                Gcol = Go_all[:, jc:jc + 1]
                Mt = MT_sb[:, jc * nb:(jc + 1) * nb]
                mx = work.tile([128, nb], bf16, tag="c")
                if jc % 11 == 10:
                    nc.vector.tensor_scalar_max(mx[:], Rbo[:], Gcol)
                else:
                    nc.gpsimd.tensor_scalar_max(mx[:], Rbo[:], Gcol)
                nc.vector.tensor_tensor(q2[:], mx[:], Mt, OP.mult)
                for ic in range(icn):
                    iw = min(128, nb - ic * 128)
                    nc.tensor.matmul(
                        vps[ic][0:iw, :], q2[:, ic * 128: ic * 128 + iw],
                        h2g_sb[:, jc * GRW + 1:(jc + 1) * GRW],
                        start=(jc == 0), stop=(jc == jcn - 1))

            # ---- Phase 6: normalize, +beta, elu(+1), log_softmax, store ----
            # wave emission: each op across all ic blocks, chains interleave
            iws = [min(128, nb - ic * 128) for ic in range(icn)]
            rcs = [spool.tile([128, 1], f32, tag="rc6", bufs=icn, name=f"rc6_{ic}")
                   for ic in range(icn)]
            for ic in range(icn):
                nc.vector.reciprocal(rcs[ic][0:iws[ic], :], vps[ic][0:iws[ic], 0:1])
            vvs = [spool.tile([128, FO], f32, tag="vv", bufs=icn, name=f"vv_{ic}")
                   for ic in range(icn)]
            for ic in range(icn):
                nc.vector.tensor_scalar_mul(vvs[ic][0:iws[ic], :],
                                            vps[ic][0:iws[ic], 1:1 + FO],
                                            rcs[ic][0:iws[ic], :])
            vts = [spool.tile([128, FO], f32, tag="vt", bufs=icn, name=f"vt_{ic}")
                   for ic in range(icn)]
            for ic in range(icn):
                nc.vector.tensor_tensor(vts[ic][0:iws[ic], :], vvs[ic][0:iws[ic], :],
                                        BETA_sb[0:iws[ic], :], OP.add)
            rs = [spool.tile([128, FO], f32, tag="r2", bufs=icn, name=f"r2_{ic}")
                  for ic in range(icn)]
            mns = [spool.tile([128, FO], f32, tag="mn2", bufs=icn, name=f"mn2_{ic}")
                   for ic in range(icn)]
            Es = [spool.tile([128, FO], f32, tag="E2", bufs=icn, name=f"Eo_{ic}")
                  for ic in range(icn)]
            for ic in range(icn):
                nc.gpsimd.tensor_scalar_max(rs[ic][0:iws[ic], :],
                                            vts[ic][0:iws[ic], :], 0.0)
                nc.vector.tensor_scalar_min(mns[ic][0:iws[ic], :],
                                            vts[ic][0:iws[ic], :], 0.0)
            for ic in range(icn):
                nc.scalar.activation(Es[ic][0:iws[ic], :], mns[ic][0:iws[ic], :],
                                     AF.Exp)
            us, nms, ses = [], [], []
            for ic in range(icn):
                u = spool.tile([128, FO], f32, tag="u", bufs=icn, name=f"u_{ic}")
                nc.vector.tensor_tensor(u[0:iws[ic], :], rs[ic][0:iws[ic], :],
                                        Es[ic][0:iws[ic], :], OP.add)
                us.append(u)
            for ic in range(icn):
                nm = spool.tile([128, 1], f32, tag="nm", bufs=icn, name=f"nm_{ic}")
                nc.vector.tensor_reduce(nm[0:iws[ic], :], us[ic][0:iws[ic], :],
                                        AX.X, OP.max, negate=True)
                nms.append(nm)
            for ic in range(icn):
                eu = spool.tile([128, FO], f32, tag="eu")
                se = spool.tile([128, 1], f32, tag="se", bufs=icn, name=f"se_{ic}")
                nc.scalar.activation(eu[0:iws[ic], :], us[ic][0:iws[ic], :],
                                     AF.Exp, bias=nms[ic][0:iws[ic], :],
                                     accum_out=se[0:iws[ic], :])
                ses.append(se)
            # group the Lns after all Exps (one activation-table switch)
            for ic in range(icn):
                iw = min(128, nb - ic * 128)
                L = spool.tile([128, 1], f32, tag="L", name=f"L_{ic}")
                nc.scalar.activation(L[0:iw, :], ses[ic][0:iw, :], AF.Ln)
                cc2 = spool.tile([128, 1], f32, tag="cc2", name=f"cc2_{ic}")
                nc.vector.tensor_tensor(cc2[0:iw, :], nms[ic][0:iw, :],
                                        L[0:iw, :], OP.subtract)
                outf = spool.tile([128, FO], f32, tag="outf", name=f"outf_{ic}")
                nc.vector.tensor_scalar_add(outf[0:iw, :], us[ic][0:iw, :],
                                            cc2[0:iw, :])
                nc.sync.dma_start(d_OUT.ap()[ic * 128: ic * 128 + iw, :],
                                  outf[0:iw, :])

    return d_OUT, dio


def build_full(nc, tc, cfg, consts, reps=1, skip_collective=False):
    dio = None
    for r in range(reps):
        sfx = "" if r == 0 else f"_r{r}"
        _, dio = build_program(nc, tc, cfg, consts,
                               skip_collective=skip_collective,
                               sfx=sfx, dio=dio)


_CACHE = {}


def run_gat(inputs, cfg=None, trace=False):
    import concourse.bacc as bacc
    import concourse.tile as tile
    from concourse.bass_utils import run_bass_kernel_spmd

    cfg = cfg or _cfg()
    in_maps, consts = host_prep(cfg=cfg, **inputs)

    nc = _CACHE.get("nc")
    if nc is None:
        nc = bacc.Bacc("TRN2", target_bir_lowering=False, debug=False,
                       num_devices=cfg["n_cores"])
        with tile.TileContext(nc) as tc:
            build_full(nc, tc, cfg, consts)
        nc.compile()
        _CACHE["nc"] = nc

    res = run_bass_kernel_spmd(nc, in_maps, list(range(cfg["n_cores"])),
                               trace=trace)
    out = np.concatenate([res.results[c]["OUT"] for c in range(cfg["n_cores"])],
                         axis=0)
    return out.astype(np.float32), res


def kernel(**inputs) -> np.ndarray:
    out, _ = run_gat(inputs)
    return out
